# revision 1
# baseline (speedup 1.0000x reference)
"""Compact bilinear pooling (count-sketch + FFT) Trainium2 kernel.

Math: for each image, y = irfft( sum_over_pixels( rfft(x_px @ S1) * rfft(x_px @ S2) ) ),
then signed-sqrt and L2 normalization.  Since rfft(x @ S) == x @ rfft(S), the
per-pixel FFTs become plain matmuls against W = rfft(S, axis=1) (precomputed on
host once per call), and since the inverse FFT is linear it is applied AFTER
spatial sum pooling, so only 4 rows per device need the inverse transform. The
inverse rfft of the pooled spectrum is computed on-device as a factored
(Cooley-Tukey, 128x64) real IDFT using two small matmul/twiddle stages.

Sharding: data-parallel over 8 NeuronCores, 4 images each; W / DFT bases are
replicated.  Everything except the rfft(S) weight prep runs on device.
"""

import os
import numpy as np

import concourse.bass as bass
import concourse.bacc as bacc
import concourse.mybir as mybir
import concourse.tile as tile
from concourse.bass_utils import run_bass_kernel_spmd

D = 8192          # projection dim
CH = 512          # input channels
HW = 196          # pixels per image (14x14)
B = 32            # batch
NCORES = 8
BPD = B // NCORES     # images per device (4)
RWS = BPD * HW        # spatial rows per device (784)
NT = 33               # 32 main freq tiles of 128 + 1 nyquist-extension tile
KEXT = NT * 128       # 4224 padded freq columns
F32 = mybir.dt.float32

# matmul storage dtype: float32r = fp32 data, full-rate PE mode
_DT_NAME = os.environ.get("CBP_MM_DTYPE", "float32r")
DT_MM = getattr(mybir.dt, _DT_NAME)
DT_B = DT_MM
_NPDT = mybir.dt.np(DT_MM)

AX = mybir.AxisListType
ALU = mybir.AluOpType
ACT = mybir.ActivationFunctionType


def _build():
    nc = bacc.Bacc(None, target_bir_lowering=False)

    xd = nc.dram_tensor("xdev", [CH, RWS], DT_MM, kind="ExternalInput")
    wall = nc.dram_tensor("wall", [4, CH, KEXT], DT_MM, kind="ExternalInput")
    cbt = nc.dram_tensor("cbt", [128, D], DT_B, kind="ExternalInput")
    sbt = nc.dram_tensor("sbt", [128, D], DT_B, kind="ExternalInput")
    cwf = nc.dram_tensor("cwf", [128, 512], F32, kind="ExternalInput")
    swf = nc.dram_tensor("swf", [128, 512], F32, kind="ExternalInput")
    selr = nc.dram_tensor("selr", [128, 4], DT_B, kind="ExternalInput")
    altr = nc.dram_tensor("altr", [1, 512], DT_B, kind="ExternalInput")
    yd = nc.dram_tensor("ydev", [BPD, D], F32, kind="ExternalOutput")

    with tile.TileContext(nc) as tc:
        with tc.tile_pool(name="singles", bufs=1) as singles:
            x_sb = singles.tile([128, 4, RWS], DT_MM)
            nc.sync.dma_start(out=x_sb, in_=xd.rearrange("(cc p) r -> p cc r", cc=4))

            # constants for the back half; loaded via the gpsimd (SWDGE) path so
            # they don't head-block the sync-ring W-tile stream.
            cb_sb = singles.tile([128, D], DT_B)
            nc.gpsimd.dma_start(out=cb_sb, in_=cbt[:, :])
            sb_sb = singles.tile([128, D], DT_B)
            nc.gpsimd.dma_start(out=sb_sb, in_=sbt[:, :])
            cwf_sb = singles.tile([128, 512], F32)
            nc.gpsimd.dma_start(out=cwf_sb, in_=cwf[:, :])
            swf_sb = singles.tile([128, 512], F32)
            nc.gpsimd.dma_start(out=swf_sb, in_=swf[:, :])
            selr_sb = singles.tile([128, 4], DT_B)
            nc.gpsimd.dma_start(out=selr_sb, in_=selr[:, :])
            alt_sb = singles.tile([1, 512], DT_B)
            nc.gpsimd.dma_start(out=alt_sb, in_=altr[:, :])

            # pooled half-spectrum, [k1=128, 4*t + img]; cols 128..131 hold the
            # nyquist products (row 0 only is meaningful there)
            P_r = singles.tile([128, NT * 4], F32)
            P_i = singles.tile([128, NT * 4], F32)

            # ---------------- phase A: projections + pooled spectral products
            wview = wall.rearrange("m (cc p) k -> p m cc k", cc=4)
            with tc.tile_pool(name="wp", bufs=2) as wp, \
                 tc.tile_pool(name="fp", bufs=2, space="PSUM") as fp, \
                 tc.tile_pool(name="cp", bufs=3) as cp, \
                 tc.tile_pool(name="scr", bufs=2) as scr:
                for t in range(NT):
                    w_sb = wp.tile([128, 4, 4, 128], DT_MM, tag="w")
                    nc.sync.dma_start(
                        out=w_sb, in_=wview[:, :, :, t * 128:(t + 1) * 128]
                    )
                    for h in range(2):
                        rsl = slice(h * 392, (h + 1) * 392)
                        fts = [
                            fp.tile([128, 392], F32, tag=f"f{m}", name=f"ft{m}")
                            for m in range(4)
                        ]
                        for m in range(4):
                            for cc in range(4):
                                nc.tensor.matmul(
                                    fts[m],
                                    lhsT=w_sb[:, m, cc, :],
                                    rhs=x_sb[:, cc, rsl],
                                    start=(cc == 0),
                                    stop=(cc == 3),
                                )
                        f2r_sb = cp.tile([128, 392], F32, tag="c0")
                        nc.scalar.copy(f2r_sb, fts[2])
                        f2i_sb = cp.tile([128, 392], F32, tag="c1")
                        nc.scalar.copy(f2i_sb, fts[3])
                        # complex product f1*f2 and per-image spatial pooling
                        m1 = scr.tile([128, 392], F32, tag="m1")
                        nc.vector.tensor_mul(m1, fts[0], f2r_sb)
                        m2 = scr.tile([128, 392], F32, tag="m2")
                        nc.vector.tensor_mul(m2, fts[1], f2i_sb)
                        m3 = scr.tile([128, 392], F32, tag="m3")
                        nc.vector.tensor_mul(m3, fts[0], f2i_sb)
                        m4 = scr.tile([128, 392], F32, tag="m4")
                        nc.vector.tensor_mul(m4, fts[1], f2r_sb)
                        d_r = scr.tile([128, 392], F32, tag="dr")
                        nc.vector.tensor_sub(d_r, m1, m2)
                        d_i = scr.tile([128, 392], F32, tag="di")
                        nc.vector.tensor_add(d_i, m3, m4)
                        for li in range(2):
                            col = 4 * t + 2 * h + li
                            sg = slice(li * HW, (li + 1) * HW)
                            nc.vector.reduce_sum(
                                P_r[:, col:col + 1], d_r[:, sg], axis=AX.X)
                            nc.vector.reduce_sum(
                                P_i[:, col:col + 1], d_i[:, sg], axis=AX.X)

            # ---------------- phase B: factored inverse rfft of pooled spectrum
            # DC bin: bases carry 2/D, k=0 needs 1/D
            nc.vector.tensor_scalar_mul(P_r[0:1, 0:4], P_r[0:1, 0:4], 0.5)
            qr = singles.tile([128, 128], DT_B)
            nc.vector.tensor_copy(qr, P_r[:, 0:128])
            qi = singles.tile([128, 128], DT_B)
            nc.vector.tensor_scalar_mul(qi, P_i[:, 0:128], -1.0)
            qrn = singles.tile([128, 128], DT_B)
            nc.vector.tensor_scalar_mul(qrn, P_r[:, 0:128], -1.0)
            pnyq = singles.tile([1, 4], DT_B)
            nc.vector.tensor_copy(pnyq, P_r[0:1, 128:132])

            ycat = singles.tile([4, D], F32)
            with tc.tile_pool(name="abp", bufs=2, space="PSUM") as abp, \
                 tc.tile_pool(name="zp", bufs=3) as zp, \
                 tc.tile_pool(name="yp", bufs=2, space="PSUM") as yp, \
                 tc.tile_pool(name="ep", bufs=1) as ep:
                for chk in range(16):
                    ks = slice(chk * 512, (chk + 1) * 512)
                    a_ps = abp.tile([128, 512], F32, tag="a")
                    b_ps = abp.tile([128, 512], F32, tag="b")
                    nc.tensor.matmul(a_ps, lhsT=qr, rhs=cb_sb[:, ks], start=True, stop=False)
                    nc.tensor.matmul(a_ps, lhsT=qi, rhs=sb_sb[:, ks], start=False, stop=True)
                    nc.tensor.matmul(b_ps, lhsT=qi, rhs=cb_sb[:, ks], start=True, stop=False)
                    nc.tensor.matmul(b_ps, lhsT=qrn, rhs=sb_sb[:, ks], start=False, stop=True)
                    z1 = zp.tile([128, 512], F32, tag="z1")
                    nc.vector.tensor_mul(z1, a_ps, cwf_sb)
                    z2 = zp.tile([128, 512], F32, tag="z2")
                    nc.vector.tensor_mul(z2, b_ps, swf_sb)
                    z = zp.tile([128, 512], DT_B, tag="z")
                    nc.vector.tensor_add(z, z1, z2)
                    y4 = yp.tile([4, 512], F32, tag="y4")
                    nc.tensor.matmul(y4, lhsT=selr_sb, rhs=z, start=True, stop=False)
                    nc.tensor.matmul(y4, lhsT=pnyq, rhs=alt_sb, start=False, stop=True)
                    nc.scalar.copy(ycat[:, ks], y4)

                # ------------ epilogue: signed sqrt + L2 normalize on [4, 8192]
                t_abs = ep.tile([4, D], F32)
                nc.scalar.activation(t_abs, ycat, ACT.Abs)
                rs = ep.tile([4, 1], F32)
                nc.vector.reduce_sum(rs, t_abs, axis=AX.X)
                # ||y_ss||^2 = sum(|y| + 1e-8) = sum|y| + D*1e-8
                e2 = ep.tile([4, 1], F32)
                nc.vector.memset(e2, float(D * 1e-8))
                nrm = ep.tile([4, 1], F32)
                nc.scalar.activation(nrm, rs, ACT.Sqrt, bias=e2)
                inv = ep.tile([4, 1], F32)
                nc.vector.reciprocal(inv, nrm)
                e1 = ep.tile([4, 1], F32)
                nc.vector.memset(e1, 1e-8)
                sgn = ep.tile([4, D], F32)
                nc.scalar.activation(sgn, ycat, ACT.Sign)
                # ss = sqrt(|y| + 1e-8), in place over t_abs
                nc.scalar.activation(t_abs, t_abs, ACT.Sqrt, bias=e1)
                # m1 = ss * sgn, in place over t_abs
                nc.vector.tensor_mul(t_abs, t_abs, sgn)
                # out = m1 * inv_norm, reusing sgn as destination
                nc.vector.tensor_scalar_mul(sgn, t_abs, inv)
                nc.sync.dma_start(out=yd[:, :], in_=sgn)
    return nc


_CACHE = {}


def _enable_axon_tracing():
    """Best-effort NTFF profiling shims for the axon agent image (test-only)."""
    if _CACHE.get("trace_shimmed"):
        return
    import sys
    import types
    try:
        from antenv.axon_hooks import get_axon_ntff_profile_hook  # noqa: F401
    except ImportError:
        try:
            from trn_agent_boot.trn_boot import _ntff_profile_via_ctypes
            hook = _ntff_profile_via_ctypes("/opt/axon/libaxon_pjrt.so")
            m = types.ModuleType("antenv.axon_hooks")
            m.get_axon_ntff_profile_hook = lambda: hook
            m.set_axon_ntff_profile_hook = lambda h: None
            sys.modules["antenv.axon_hooks"] = m
        except Exception as e:  # pragma: no cover
            print("tracing shim unavailable:", e)
    try:
        import concourse.bass_utils as bu
        bu.upload_artifacts = lambda tmpdir: f"local://{tmpdir}"
    except Exception as e:  # pragma: no cover
        print("upload shim failed:", e)
    _CACHE["trace_shimmed"] = True


def _host_consts():
    if "consts" in _CACHE:
        return _CACHE["consts"]
    k1 = np.arange(128, dtype=np.int64)[:, None]
    n = np.arange(D, dtype=np.int64)[None, :]
    ang = 2.0 * np.pi * ((k1 * n) % D) / D
    cbt = (np.cos(ang) * (2.0 / D)).astype(_NPDT)
    sbt = (np.sin(ang) * (2.0 / D)).astype(_NPDT)

    p = np.arange(128, dtype=np.int64)[:, None]
    j = np.arange(512, dtype=np.int64)[None, :]
    ang2 = 2.0 * np.pi * ((p // 4) * (j % 64) % 64) / 64.0
    cwf = np.cos(ang2).astype(np.float32)
    swf = np.sin(ang2).astype(np.float32)

    sel = np.zeros((128, 4), np.float32)
    sel[np.arange(128), np.arange(128) % 4] = 1.0
    l4 = np.zeros((4, 128), np.float32)
    l4[np.arange(128) % 4, np.arange(128)] = 1.0
    alt = (((-1.0) ** np.arange(512)) / D).astype(_NPDT)[None, :]
    _CACHE["consts"] = (cbt, sbt, cwf, swf, sel, l4, alt)
    return _CACHE["consts"]


def kernel(x, S1, S2):
    x = np.ascontiguousarray(x, dtype=np.float32)
    S1 = np.asarray(S1, dtype=np.float32)
    S2 = np.asarray(S2, dtype=np.float32)

    W1 = np.fft.rfft(S1.astype(np.float64), axis=1)  # [512, 4097]
    W2 = np.fft.rfft(S2.astype(np.float64), axis=1)
    wall = np.zeros((4, CH, KEXT), _NPDT)
    wall[0, :, :D // 2] = W1.real[:, :D // 2]
    wall[1, :, :D // 2] = W1.imag[:, :D // 2]
    wall[2, :, :D // 2] = W2.real[:, :D // 2]
    wall[3, :, :D // 2] = W2.imag[:, :D // 2]
    wall[0, :, D // 2] = W1.real[:, D // 2]  # nyquist -> tile 32, col 0
    wall[2, :, D // 2] = W2.real[:, D // 2]

    cbt, sbt, cwf, swf, sel, l4, alt = _host_consts()

    if "nc" not in _CACHE:
        nc = _build()
        nc.finalize()
        _CACHE["nc"] = nc
    nc = _CACHE["nc"]

    common = {
        "wall": wall, "cbt": cbt, "sbt": sbt, "cwf": cwf, "swf": swf,
        "selr": sel.astype(_NPDT), "altr": alt,
    }
    in_maps = []
    for d in range(NCORES):
        xdev = np.ascontiguousarray(
            x[d * BPD:(d + 1) * BPD].transpose(1, 0, 2, 3).reshape(CH, RWS)
        ).astype(_NPDT)
        in_maps.append({"xdev": xdev, **common})

    trace = bool(int(os.environ.get("CBP_TRACE", "0")))
    if trace:
        _enable_axon_tracing()
    res = run_bass_kernel_spmd(nc, in_maps, list(range(NCORES)), trace=trace)
    _CACHE["last_results"] = res
    out = np.concatenate(
        [np.asarray(res.results[d]["ydev"]) for d in range(NCORES)], axis=0
    )
    return out.astype(np.float32)



# revision 6
# speedup vs baseline: 1.6032x; 1.6032x over previous
"""Compact bilinear pooling (count-sketch + FFT) Trainium2 kernel.

Math: for each image, y = irfft( sum_over_pixels( rfft(x_px @ S1) * rfft(x_px @ S2) ) ),
then signed-sqrt and L2 normalization.  Since rfft(x @ S) == x @ rfft(S), the
per-pixel FFTs become plain matmuls against W = rfft(S, axis=1) (precomputed on
host once per call), and since the inverse FFT is linear it is applied AFTER
spatial sum pooling, so only 4 rows per device need the inverse transform. The
inverse rfft of the pooled spectrum is computed on-device as a factored
(Cooley-Tukey, 128x64) real IDFT using two small matmul/twiddle stages.

v2: the complex product + spatial pooling is done with fused
tensor_tensor_reduce (DVE) / scalar_tensor_tensor (GpSimd) ops reading the
matmul PSUM tiles directly (one fused multiply+pool op per image/term), the
W-tile stream is split across two DMA queues with deeper prefetch so the PE
never stalls (stalls reset the tensor-engine pstate ramp), and the PSUM
evacuation copies are gone.

Sharding: data-parallel over 8 NeuronCores, 4 images each; W / DFT bases are
replicated.  Everything except the rfft(S) weight prep runs on device.
"""

import os
import numpy as np

import concourse.bass as bass
import concourse.bacc as bacc
import concourse.mybir as mybir
import concourse.tile as tile
from concourse.bass_utils import run_bass_kernel_spmd

D = 8192          # projection dim
CH = 512          # input channels
HW = 196          # pixels per image (14x14)
B = 32            # batch
NCORES = 8
BPD = B // NCORES     # images per device (4)
RWS = BPD * HW        # spatial rows per device (784)
NT = 33               # 32 main freq tiles of 128 + 1 nyquist-extension tile
KEXT = NT * 128       # 4224 padded freq columns
F32 = mybir.dt.float32

# matmul storage dtype: float32r = fp32 data, full-rate PE mode
_DT_NAME = os.environ.get("CBP_MM_DTYPE", "float32r")
DT_MM = getattr(mybir.dt, _DT_NAME)
DT_B = DT_MM
_NPDT = mybir.dt.np(DT_MM)

# how many images' cross-terms (ri/ir) go to GpSimd (rest on DVE)
GPS_IMGS = int(os.environ.get("CBP_GPS_IMGS", "2"))
WP_BUFS = int(os.environ.get("CBP_WP_BUFS", "3"))

AX = mybir.AxisListType
ALU = mybir.AluOpType
ACT = mybir.ActivationFunctionType


def _build():
    nc = bacc.Bacc(None, target_bir_lowering=False)

    xd = nc.dram_tensor("xdev", [CH, RWS], DT_MM, kind="ExternalInput")
    wall = nc.dram_tensor("wall", [4, CH, KEXT], DT_MM, kind="ExternalInput")
    cbt = nc.dram_tensor("cbt", [128, D], DT_B, kind="ExternalInput")
    sbt = nc.dram_tensor("sbt", [128, D], DT_B, kind="ExternalInput")
    cwf = nc.dram_tensor("cwf", [128, 512], F32, kind="ExternalInput")
    swf = nc.dram_tensor("swf", [128, 512], F32, kind="ExternalInput")
    selr = nc.dram_tensor("selr", [128, 4], DT_B, kind="ExternalInput")
    altr = nc.dram_tensor("altr", [1, 512], DT_B, kind="ExternalInput")
    yd = nc.dram_tensor("ydev", [BPD, D], F32, kind="ExternalOutput")

    with tile.TileContext(nc) as tc:
        with tc.tile_pool(name="singles", bufs=1) as singles:
            x_sb = singles.tile([128, 4, RWS], DT_MM)
            xv = xd.rearrange("(cc p) r -> p cc r", cc=4)
            nc.sync.dma_start(out=x_sb[:, 0:2], in_=xv[:, 0:2])
            nc.scalar.dma_start(out=x_sb[:, 2:4], in_=xv[:, 2:4])

            # constants for the back half; loaded via the gpsimd (SWDGE) path
            # so they don't head-block the sync/scalar W-tile streams.
            cb_sb = singles.tile([128, D], DT_B)
            nc.gpsimd.dma_start(out=cb_sb, in_=cbt[:, :])
            sb_sb = singles.tile([128, D], DT_B)
            nc.gpsimd.dma_start(out=sb_sb, in_=sbt[:, :])
            cwf_sb = singles.tile([128, 512], F32)
            nc.gpsimd.dma_start(out=cwf_sb, in_=cwf[:, :])
            swf_sb = singles.tile([128, 512], F32)
            nc.gpsimd.dma_start(out=swf_sb, in_=swf[:, :])
            selr_sb = singles.tile([128, 4], DT_B)
            nc.gpsimd.dma_start(out=selr_sb, in_=selr[:, :])
            alt_sb = singles.tile([1, 512], DT_B)
            nc.gpsimd.dma_start(out=alt_sb, in_=altr[:, :])

            # fused product+pool accumulators, [k1=128, 4*t + img]
            # P_r = Prr - Pii ; P_i = Pri + Pir  (combined once at the end)
            Prr = singles.tile([128, NT * 4], F32)
            Pnii = singles.tile([128, NT * 4], F32)
            Pri = singles.tile([128, NT * 4], F32)
            Pir = singles.tile([128, NT * 4], F32)
            P_r = singles.tile([128, NT * 4], F32)
            P_i = singles.tile([128, NT * 4], F32)

            # ---------------- phase A: projections + pooled spectral products
            wview = wall.rearrange("m (cc p) k -> p m cc k", cc=4)
            with tc.tile_pool(name="wp", bufs=WP_BUFS) as wp, \
                 tc.tile_pool(name="fp", bufs=2, space="PSUM") as fp, \
                 tc.tile_pool(name="scr", bufs=3) as scr:
                for t in range(NT):
                    w_sb = wp.tile([128, 4, 4, 128], DT_MM, tag="w")
                    # split the 1MB tile across two DMA queues
                    nc.sync.dma_start(
                        out=w_sb[:, 0:2], in_=wview[:, 0:2, :, t * 128:(t + 1) * 128]
                    )
                    nc.scalar.dma_start(
                        out=w_sb[:, 2:4], in_=wview[:, 2:4, :, t * 128:(t + 1) * 128]
                    )
                    for h in range(2):
                        rsl = slice(h * 392, (h + 1) * 392)
                        fts = [
                            fp.tile([128, 392], F32, tag=f"f{m}", name=f"ft{m}")
                            for m in range(4)
                        ]
                        for m in range(4):
                            for cc in range(4):
                                nc.tensor.matmul(
                                    fts[m],
                                    lhsT=w_sb[:, m, cc, :],
                                    rhs=x_sb[:, cc, rsl],
                                    start=(cc == 0),
                                    stop=(cc == 3),
                                )
                        # ISA allows only one PSUM operand per instruction:
                        # evacuate f2r/f2i to SBUF on the (otherwise idle)
                        # scalar engine, then fuse product+pool reading
                        # f1r/f1i straight from PSUM.
                        f2r_sb = scr.tile([128, 392], F32, tag="c0")
                        nc.scalar.copy(f2r_sb, fts[2])
                        f2i_sb = scr.tile([128, 392], F32, tag="c1")
                        nc.scalar.copy(f2i_sb, fts[3])
                        for li in range(2):
                            img = 2 * h + li
                            col = 4 * t + img
                            sg = slice(li * HW, (li + 1) * HW)
                            so = scr.tile([128, HW], F32, tag="so", name="so")
                            nc.vector.scalar_tensor_tensor(
                                out=so, in0=fts[0][:, sg], scalar=1.0,
                                in1=f2r_sb[:, sg], op0=ALU.mult, op1=ALU.mult,
                                accum_out=Prr[:, col:col + 1],
                            )
                            so2 = scr.tile([128, HW], F32, tag="so2", name="so2")
                            nc.vector.scalar_tensor_tensor(
                                out=so2, in0=fts[1][:, sg], scalar=-1.0,
                                in1=f2i_sb[:, sg], op0=ALU.mult, op1=ALU.mult,
                                accum_out=Pnii[:, col:col + 1],
                            )
                            if img >= BPD - GPS_IMGS:
                                so3 = scr.tile([128, HW], F32, tag="so3")
                                nc.gpsimd.scalar_tensor_tensor(
                                    out=so3, in0=fts[0][:, sg], scalar=1.0,
                                    in1=f2i_sb[:, sg], op0=ALU.mult, op1=ALU.mult,
                                    accum_out=Pri[:, col:col + 1],
                                )
                                so4 = scr.tile([128, HW], F32, tag="so4")
                                nc.gpsimd.scalar_tensor_tensor(
                                    out=so4, in0=fts[1][:, sg], scalar=1.0,
                                    in1=f2r_sb[:, sg], op0=ALU.mult, op1=ALU.mult,
                                    accum_out=Pir[:, col:col + 1],
                                )
                            else:
                                so3 = scr.tile([128, HW], F32, tag="so3", name="so3")
                                nc.vector.scalar_tensor_tensor(
                                    out=so3, in0=fts[0][:, sg], scalar=1.0,
                                    in1=f2i_sb[:, sg], op0=ALU.mult, op1=ALU.mult,
                                    accum_out=Pri[:, col:col + 1],
                                )
                                so4 = scr.tile([128, HW], F32, tag="so4", name="so4")
                                nc.vector.scalar_tensor_tensor(
                                    out=so4, in0=fts[1][:, sg], scalar=1.0,
                                    in1=f2r_sb[:, sg], op0=ALU.mult, op1=ALU.mult,
                                    accum_out=Pir[:, col:col + 1],
                                )

                # combine partial pools: P_r = Prr - Pii, P_i = Pri + Pir
                nc.vector.tensor_add(P_r, Prr, Pnii)   # Pnii already negated
                nc.vector.tensor_add(P_i, Pri, Pir)

            # ---------------- phase B: factored inverse rfft of pooled spectrum
            # DC bin: bases carry 2/D, k=0 needs 1/D
            nc.vector.tensor_scalar_mul(P_r[0:1, 0:4], P_r[0:1, 0:4], 0.5)
            qr = singles.tile([128, 128], DT_B)
            nc.vector.tensor_copy(qr, P_r[:, 0:128])
            qi = singles.tile([128, 128], DT_B)
            nc.vector.tensor_scalar_mul(qi, P_i[:, 0:128], -1.0)
            qrn = singles.tile([128, 128], DT_B)
            nc.vector.tensor_scalar_mul(qrn, P_r[:, 0:128], -1.0)
            pnyq = singles.tile([1, 4], DT_B)
            nc.vector.tensor_copy(pnyq, P_r[0:1, 128:132])

            ycat = singles.tile([4, D], F32)
            with tc.tile_pool(name="abp", bufs=2, space="PSUM") as abp, \
                 tc.tile_pool(name="zp", bufs=3) as zp, \
                 tc.tile_pool(name="yp", bufs=2, space="PSUM") as yp, \
                 tc.tile_pool(name="ep", bufs=1) as ep:
                for chk in range(16):
                    ks = slice(chk * 512, (chk + 1) * 512)
                    a_ps = abp.tile([128, 512], F32, tag="a")
                    b_ps = abp.tile([128, 512], F32, tag="b")
                    nc.tensor.matmul(a_ps, lhsT=qr, rhs=cb_sb[:, ks], start=True, stop=False)
                    nc.tensor.matmul(a_ps, lhsT=qi, rhs=sb_sb[:, ks], start=False, stop=True)
                    nc.tensor.matmul(b_ps, lhsT=qi, rhs=cb_sb[:, ks], start=True, stop=False)
                    nc.tensor.matmul(b_ps, lhsT=qrn, rhs=sb_sb[:, ks], start=False, stop=True)
                    z1 = zp.tile([128, 512], F32, tag="z1")
                    nc.vector.tensor_mul(z1, a_ps, cwf_sb)
                    z2 = zp.tile([128, 512], F32, tag="z2")
                    nc.vector.tensor_mul(z2, b_ps, swf_sb)
                    z = zp.tile([128, 512], DT_B, tag="z")
                    nc.vector.tensor_add(z, z1, z2)
                    y4 = yp.tile([4, 512], F32, tag="y4")
                    nc.tensor.matmul(y4, lhsT=selr_sb, rhs=z, start=True, stop=False)
                    nc.tensor.matmul(y4, lhsT=pnyq, rhs=alt_sb, start=False, stop=True)
                    nc.scalar.copy(ycat[:, ks], y4)

                # ------------ epilogue: signed sqrt + L2 normalize on [4, 8192]
                t_abs = ep.tile([4, D], F32)
                nc.scalar.activation(t_abs, ycat, ACT.Abs)
                rs = ep.tile([4, 1], F32)
                nc.vector.reduce_sum(rs, t_abs, axis=AX.X)
                # ||y_ss||^2 = sum(|y| + 1e-8) = sum|y| + D*1e-8
                e2 = ep.tile([4, 1], F32)
                nc.vector.memset(e2, float(D * 1e-8))
                nrm = ep.tile([4, 1], F32)
                nc.scalar.activation(nrm, rs, ACT.Sqrt, bias=e2)
                inv = ep.tile([4, 1], F32)
                nc.vector.reciprocal(inv, nrm)
                e1 = ep.tile([4, 1], F32)
                nc.vector.memset(e1, 1e-8)
                sgn = ep.tile([4, D], F32)
                nc.scalar.activation(sgn, ycat, ACT.Sign)
                # ss = sqrt(|y| + 1e-8), in place over t_abs
                nc.scalar.activation(t_abs, t_abs, ACT.Sqrt, bias=e1)
                # m1 = ss * sgn, in place over t_abs
                nc.vector.tensor_mul(t_abs, t_abs, sgn)
                # out = m1 * inv_norm, reusing sgn as destination
                nc.vector.tensor_scalar_mul(sgn, t_abs, inv)
                nc.sync.dma_start(out=yd[:, :], in_=sgn)
    return nc


_CACHE = {}


def _enable_axon_tracing():
    """Best-effort NTFF profiling shims for the axon agent image (test-only)."""
    if _CACHE.get("trace_shimmed"):
        return
    import sys
    import types
    try:
        from antenv.axon_hooks import get_axon_ntff_profile_hook  # noqa: F401
    except ImportError:
        try:
            from trn_agent_boot.trn_boot import _ntff_profile_via_ctypes
            hook = _ntff_profile_via_ctypes("/opt/axon/libaxon_pjrt.so")
            m = types.ModuleType("antenv.axon_hooks")
            m.get_axon_ntff_profile_hook = lambda: hook
            m.set_axon_ntff_profile_hook = lambda h: None
            sys.modules["antenv.axon_hooks"] = m
        except Exception as e:  # pragma: no cover
            print("tracing shim unavailable:", e)
    try:
        import concourse.bass_utils as bu
        bu.upload_artifacts = lambda tmpdir: f"local://{tmpdir}"
    except Exception as e:  # pragma: no cover
        print("upload shim failed:", e)
    _CACHE["trace_shimmed"] = True


def _host_consts():
    if "consts" in _CACHE:
        return _CACHE["consts"]
    k1 = np.arange(128, dtype=np.int64)[:, None]
    n = np.arange(D, dtype=np.int64)[None, :]
    ang = 2.0 * np.pi * ((k1 * n) % D) / D
    cbt = (np.cos(ang) * (2.0 / D)).astype(_NPDT)
    sbt = (np.sin(ang) * (2.0 / D)).astype(_NPDT)

    p = np.arange(128, dtype=np.int64)[:, None]
    j = np.arange(512, dtype=np.int64)[None, :]
    ang2 = 2.0 * np.pi * ((p // 4) * (j % 64) % 64) / 64.0
    cwf = np.cos(ang2).astype(np.float32)
    swf = np.sin(ang2).astype(np.float32)

    sel = np.zeros((128, 4), np.float32)
    sel[np.arange(128), np.arange(128) % 4] = 1.0
    l4 = np.zeros((4, 128), np.float32)
    l4[np.arange(128) % 4, np.arange(128)] = 1.0
    alt = (((-1.0) ** np.arange(512)) / D).astype(_NPDT)[None, :]
    _CACHE["consts"] = (cbt, sbt, cwf, swf, sel, l4, alt)
    return _CACHE["consts"]


def kernel(x, S1, S2):
    x = np.ascontiguousarray(x, dtype=np.float32)
    S1 = np.asarray(S1, dtype=np.float32)
    S2 = np.asarray(S2, dtype=np.float32)

    W1 = np.fft.rfft(S1.astype(np.float64), axis=1)  # [512, 4097]
    W2 = np.fft.rfft(S2.astype(np.float64), axis=1)
    wall = np.zeros((4, CH, KEXT), _NPDT)
    wall[0, :, :D // 2] = W1.real[:, :D // 2]
    wall[1, :, :D // 2] = W1.imag[:, :D // 2]
    wall[2, :, :D // 2] = W2.real[:, :D // 2]
    wall[3, :, :D // 2] = W2.imag[:, :D // 2]
    wall[0, :, D // 2] = W1.real[:, D // 2]  # nyquist -> tile 32, col 0
    wall[2, :, D // 2] = W2.real[:, D // 2]

    cbt, sbt, cwf, swf, sel, l4, alt = _host_consts()

    if "nc" not in _CACHE:
        nc = _build()
        nc.finalize()
        _CACHE["nc"] = nc
    nc = _CACHE["nc"]

    common = {
        "wall": wall, "cbt": cbt, "sbt": sbt, "cwf": cwf, "swf": swf,
        "selr": sel.astype(_NPDT), "altr": alt,
    }
    in_maps = []
    for d in range(NCORES):
        xdev = np.ascontiguousarray(
            x[d * BPD:(d + 1) * BPD].transpose(1, 0, 2, 3).reshape(CH, RWS)
        ).astype(_NPDT)
        in_maps.append({"xdev": xdev, **common})

    trace = bool(int(os.environ.get("CBP_TRACE", "0")))
    if trace:
        _enable_axon_tracing()
    res = run_bass_kernel_spmd(nc, in_maps, list(range(NCORES)), trace=trace)
    _CACHE["last_results"] = res
    out = np.concatenate(
        [np.asarray(res.results[d]["ydev"]) for d in range(NCORES)], axis=0
    )
    return out.astype(np.float32)
